# revision 5
# baseline (speedup 1.0000x reference)
"""Trainium2 Bass kernel for a single nGPT-style attention head.

Computation (see reference): fused QKV projection, RoPE over the full head
dim, L2-normalize q/k scaled by sqk, causal SDPA with scale sqrt(d_model).

Sharding: data-parallel over batch - 8 batch elements, one per NeuronCore.

Per-core layout: everything transposed, [d, t] with head/feature dim on
SBUF partitions. Software-pipelined per-block structure (TB=512 token
blocks): block j runs QKV matmuls, norms, RoPE, while the ATTENTION of
query block j-1 is interleaved strip-by-strip between the QKV matmul
groups so the scores-exp latency hides behind projection matmuls.

Engine assignment (keeps the scalar engine's activation table resident on
the exp-containing set for the whole kernel - a single ACT_TABLE_LOAD):
  PE:  QKV matmuls, rotate-half (permutation matmul), v transpose
       (identity matmul), transposed norm reductions (sq^T @ ones ->
       per-token partition layout), scores, attn@v, denominator reduce.
  ACT: psum->sbuf copies and the scores exp, with the 1/||k|| fold done
       via the per-partition scale AP operand.
  DVE: squares, Quake-style rsqrt (bitcast + shift seed + 2 Newton
       steps), RoPE elementwise chain, causal triangle mask,
       reciprocal_approx_fast, final out normalize.
  GpSimd: denominator strip accumulation, inv-norm partition broadcasts.
"""

import numpy as np
import ml_dtypes

import concourse.bass as bass
import concourse.tile as tile
from concourse import bacc, mybir
from concourse.bass import ts, ds
from concourse.bass_utils import run_bass_kernel_spmd

# Surface compile-hook exceptions (the PJRT bridge swallows tracebacks).
try:
    import traceback
    import libneuronxla as _lnx

    if not getattr(_lnx, "_err_wrapped", False):
        _orig_cc = _lnx.neuronx_cc

        def _cc_wrapper(*a, **kw):
            try:
                return _orig_cc(*a, **kw)
            except BaseException:
                traceback.print_exc()
                raise

        _lnx.neuronx_cc = _cc_wrapper
        _lnx._err_wrapped = True
except Exception:
    pass

AFT = mybir.ActivationFunctionType
ALU = mybir.AluOpType
F32 = mybir.dt.float32
BF16 = mybir.dt.bfloat16
I32 = mybir.dt.int32

B, T_FULL, C, D = 8, 2048, 1024, 128
ROPE_BASE = 10000.0
P = 128
TB = 512            # token block (tq block width = one PSUM bank of f32)
NCO = C // P        # contraction chunks for the QKV projection
H = P // 2


def build_nc(T=T_FULL, num_devices=8):
    from contextlib import ExitStack
    NTB = T // TB
    NKT = T // P
    nc = bacc.Bacc("TRN2", target_bir_lowering=False, debug=False,
                   num_devices=num_devices)

    xT = nc.dram_tensor("xT", [P, NTB, NCO, TB], BF16,
                        kind="ExternalInput").ap()
    WT = nc.dram_tensor("WT", [C, 3 * D], BF16, kind="ExternalInput").ap()
    cosF = nc.dram_tensor("cosF", [P, T], BF16, kind="ExternalInput").ap()
    sinF = nc.dram_tensor("sinF", [P, T], BF16, kind="ExternalInput").ap()
    tri = nc.dram_tensor("tri", [P, P], BF16, kind="ExternalInput").ap()
    idn = nc.dram_tensor("idn", [P, P], BF16, kind="ExternalInput").ap()
    smat = nc.dram_tensor("smat", [P, P], BF16, kind="ExternalInput").ap()
    onb = nc.dram_tensor("onb", [P, 1], BF16, kind="ExternalInput").ap()
    sqk232 = nc.dram_tensor("sqk232", [D, 1], F32, kind="ExternalInput").ap()
    outT = nc.dram_tensor("outT", [D, T], F32, kind="ExternalOutput").ap()

    WT_t = WT.rearrange("(co p) d -> p co d", p=P)

    with tile.TileContext(nc) as tc:
        with ExitStack() as ctx:
            const = ctx.enter_context(tc.tile_pool(name="const", bufs=1))
            work = ctx.enter_context(tc.tile_pool(name="work", bufs=2))
            xpool = ctx.enter_context(tc.tile_pool(name="xpool", bufs=NTB))
            expool = ctx.enter_context(tc.tile_pool(name="expool", bufs=3))
            ps_big = ctx.enter_context(
                tc.tile_pool(name="ps_big", bufs=3, space="PSUM"))
            ps_sc = ctx.enter_context(
                tc.tile_pool(name="ps_sc", bufs=2, space="PSUM"))
            ps_o = ctx.enter_context(
                tc.tile_pool(name="ps_o", bufs=1, space="PSUM"))
            ps_sm = ctx.enter_context(
                tc.tile_pool(name="ps_sm", bufs=2, space="PSUM"))

            # constants (small first, then the big streams in need order)
            tri_sb = const.tile([P, P], BF16)
            nc.sync.dma_start(tri_sb, tri)
            idn_sb = const.tile([P, P], BF16)
            nc.sync.dma_start(idn_sb, idn)
            smat_sb = const.tile([P, P], BF16)
            nc.sync.dma_start(smat_sb, smat)
            onb_sb = const.tile([P, 1], BF16)
            nc.sync.dma_start(onb_sb, onb)
            sqk_sb = const.tile([D, 1], F32)
            nc.sync.dma_start(sqk_sb, sqk232)
            wt = const.tile([P, NCO, 3 * D], BF16)
            nc.sync.dma_start(wt, WT_t)

            xts = []
            for j in range(NTB):
                xt = xpool.tile([P, NCO, TB], BF16, tag="xt", name=f"xt{j}")
                nc.sync.dma_start(xt, xT[:, j])
                xts.append(xt)
                if j == 0:
                    cos_sb = const.tile([P, T], BF16)
                    nc.sync.dma_start(cos_sb, cosF)
                    sin_sb = const.tile([P, T], BF16)
                    nc.sync.dma_start(sin_sb, sinF)

            qk = const.tile([P, 2 * T], BF16)    # roped q^T | roped-raw k^T
            vt = const.tile([P, NKT, P], BF16)   # v tiles [tk, e]
            invk = const.tile([P, NKT], F32)     # 1/||k|| per k-strip column

            # ---- pending attention work queue (strips of block J=j-1) ----
            pend = []

            def drain(n):
                for _ in range(min(n, len(pend))):
                    pend.pop(0)()

            def make_att(J):
                """Queue attention for query block J as strip closures."""
                q_blk = qk[:, ts(J, TB)]
                po = ps_o.tile([P, TB], F32, tag="po", name=f"po{J}")
                acc = work.tile([P, TB], F32, tag="acc", name=f"acc{J}")
                nstr = 4 * (J + 1)
                exs = {}

                def emit_scores(i):
                    dr = i - 4 * J
                    off = P * dr if dr >= 0 else 0
                    w = TB - off
                    sc = ps_sc.tile([P, TB], F32, tag="sc",
                                    name=f"sc{J}i{i}")
                    nc.tensor.matmul(sc[:, ds(off, w)],
                                     qk[:, ds(T + P * i, P)],
                                     q_blk[:, ds(off, w)],
                                     start=True, stop=True)
                    ex = expool.tile([P, TB], BF16, tag="ex",
                                     name=f"ex{J}i{i}")
                    nc.scalar.activation(ex[:, ds(off, w)],
                                         sc[:, ds(off, w)], AFT.Exp,
                                         scale=invk[:, ds(i, 1)])
                    if dr >= 0:
                        nc.vector.tensor_mul(ex[:, ds(off, P)],
                                             ex[:, ds(off, P)], tri_sb)
                    exs[i] = (ex, off)

                def emit_av(i):
                    ex, off = exs.pop(i)
                    w = TB - off
                    nc.tensor.matmul(po[:, ds(off, w)], vt[:, i],
                                     ex[:, ds(off, w)],
                                     start=(i == 0), stop=(i == nstr - 1))
                    if i == 0:
                        nc.gpsimd.tensor_copy(acc, ex)
                    else:
                        nc.gpsimd.tensor_add(acc[:, ds(off, w)],
                                             acc[:, ds(off, w)],
                                             ex[:, ds(off, w)])

                def fin():
                    with nc.named_scope(f"fin{J}"):
                        accb = work.tile([P, TB], BF16, tag="accb")
                        nc.gpsimd.tensor_copy(accb, acc)
                        red = ps_sm.tile([1, TB], F32, tag="small",
                                         name=f"red{J}")
                        nc.tensor.matmul(red, onb_sb, accb,
                                         start=True, stop=True)
                        reds = work.tile([1, TB], F32, tag="reds")
                        nc.scalar.activation(reds, red, AFT.Copy)
                        invd = work.tile([1, TB], F32, tag="invd")
                        nc.vector.reciprocal_approx_fast(out=invd, in_=reds)
                        bcd = work.tile([P, TB], F32, tag="bcd")
                        nc.gpsimd.partition_broadcast(bcd, invd)
                        ob = work.tile([P, TB], F32, tag="ob")
                        nc.vector.tensor_mul(ob, po, bcd)
                        nc.sync.dma_start(outT[:, ts(J, TB)], ob)

                def strip(i, J=J):
                    def run():
                        with nc.named_scope(f"att{J}s{i}"):
                            emit_scores(i)
                            if i > 0:
                                emit_av(i - 1)
                            if i == nstr - 1:
                                emit_av(i)
                                fin()
                    return run

                for i in range(nstr):
                    pend.append(strip(i))

            for j in range(NTB):
                xt = xts[j]
                # ---------------- QKV projection + squares ----------------
                with nc.named_scope(f"qkv{j}"):
                    sq = work.tile([P, 2, TB], BF16, tag="sq")
                    for g in range(3):
                        ps = ps_big.tile([P, TB], F32, tag="big",
                                         name=f"qkv{j}g{g}")
                        for co in range(NCO):
                            nc.tensor.matmul(
                                ps, wt[:, co, ts(g, D)], xt[:, co],
                                start=(co == 0), stop=(co == NCO - 1))
                            if co == 3:
                                drain(1)
                        if g < 2:
                            dst = qk[:, ds(g * T + j * TB, TB)]
                            nc.scalar.activation(dst, ps, AFT.Copy)
                            nc.vector.tensor_mul(sq[:, g], dst, dst)
                        else:
                            vst = work.tile([P, TB], BF16, tag="vst")
                            nc.scalar.activation(vst, ps, AFT.Copy)
                        drain(1)

                    # transposed norm reductions: nrm[t_local, g*4+c]
                    nrm = ps_sm.tile([P, 8], F32, tag="small",
                                     name=f"nrm{j}")
                    for g in range(2):
                        for c in range(4):
                            nc.tensor.matmul(
                                nrm[:, ds(g * 4 + c, 1)],
                                sq[:, g, ts(c, P)], onb_sb,
                                start=True, stop=True)
                    nrs = work.tile([P, 8], F32, tag="nrs")
                    nc.scalar.activation(nrs, nrm, AFT.Copy)
                    drain(1)

                    # v transpose via identity matmul
                    for c in range(4):
                        tp = ps_sm.tile([P, P], BF16, tag="small",
                                        name=f"vtp{j}c{c}")
                        nc.tensor.transpose(tp, vst[:, ts(c, P)], idn_sb)
                        nc.vector.tensor_copy(vt[:, 4 * j + c], tp)
                        drain(1)

                # -------- rsqrt of norms (Quake seed + 2 Newton) on DVE ----
                with nc.named_scope(f"nrm{j}"):
                    hg = work.tile([P, 8], I32, tag="hg")
                    nc.vector.tensor_scalar(
                        out=hg, in0=nrs.bitcast(I32), scalar1=1,
                        scalar2=None, op0=ALU.logical_shift_right)
                    nc.vector.tensor_scalar(
                        out=hg, in0=hg, scalar1=-1.0,
                        scalar2=float(0x5F3759DF), op0=ALU.mult, op1=ALU.add)
                    y = work.tile([P, 8], F32, tag="y")
                    nc.vector.tensor_copy(y, hg.bitcast(F32))
                    a = work.tile([P, 8], F32, tag="a")
                    nc.vector.tensor_mul(a, y, y)
                    nc.vector.tensor_mul(a, a, nrs)
                    nc.vector.tensor_scalar(out=a, in0=a, scalar1=-0.5,
                                            scalar2=1.5, op0=ALU.mult,
                                            op1=ALU.add)
                    nc.vector.tensor_mul(y, y, a)
                    nc.vector.tensor_mul(a, y, y)
                    nc.vector.tensor_mul(a, a, nrs)
                    nc.vector.tensor_scalar(out=a, in0=a, scalar1=-0.5,
                                            scalar2=1.5, op0=ALU.mult,
                                            op1=ALU.add)
                    yqb = work.tile([P, 4], BF16, tag="yqb")
                    nc.vector.tensor_mul(yqb, y[:, 0:4], a[:, 0:4])
                    nc.vector.tensor_mul(invk[:, ds(4 * j, 4)],
                                         y[:, 4:8], a[:, 4:8])
                    drain(1)

                    # 1/||q||: transpose each [128,1] column to a
                    # partition-0 row of a [1,TB] staging tile, broadcast
                    # across partitions on GpSimd.
                    iqt = work.tile([1, TB], F32, tag="iqt")
                    for c in range(4):
                        tq = ps_sm.tile([1, P], BF16, tag="small",
                                        name=f"tq{j}c{c}")
                        nc.tensor.transpose(tq, yqb[:, ds(c, 1)], idn_sb)
                        nc.vector.tensor_copy(iqt[:, ts(c, P)], tq)
                    bcq = work.tile([P, TB], F32, tag="bcq")
                    nc.gpsimd.partition_broadcast(bcq, iqt)
                    drain(1)

                # ------------------------- RoPE ---------------------------
                with nc.named_scope(f"rope{j}"):
                    ch_t = ds(j * TB, TB)
                    for part in range(2):  # 0 = q, 1 = k
                        chq = ds(part * T + j * TB, TB)
                        rot = ps_big.tile([P, TB], F32, tag="big",
                                          name=f"rot{j}p{part}")
                        nc.tensor.matmul(rot, smat_sb, qk[:, chq],
                                         start=True, stop=True)
                        t2 = work.tile([P, TB], BF16, tag="t2")
                        nc.vector.tensor_mul(t2, rot, sin_sb[:, ch_t])
                        t1 = work.tile([P, TB], BF16, tag="t1")
                        nc.vector.tensor_mul(t1, qk[:, chq], cos_sb[:, ch_t])
                        if part == 0:
                            nc.vector.tensor_add(t1, t1, t2)
                            nc.vector.scalar_tensor_tensor(
                                out=qk[:, chq], in0=t1, scalar=sqk_sb,
                                in1=bcq, op0=ALU.mult, op1=ALU.mult)
                        else:
                            nc.vector.tensor_add(qk[:, chq], t1, t2)
                        drain(1)

                # queue this block's attention; last block drains everything
                make_att(j)
                if j == NTB - 1:
                    drain(len(pend))

    nc.compile()
    return nc


def _host_tables(T):
    d = D
    inv_freq = 1.0 / (ROPE_BASE ** (np.arange(0, d, 2, dtype=np.float64) / d))
    t = np.arange(T, dtype=np.float64)
    freqs = np.outer(inv_freq, t)                 # [d/2, T]
    emb = np.concatenate([freqs, freqs], axis=0)  # [d, T]
    cos1 = np.cos(emb)
    sin1 = np.sin(emb)
    # rotate_half sign folded into the sin table: the device rot is a plain
    # half-swap permutation; sin rows 0:d/2 carry the minus sign.
    sin1[: d // 2, :] *= -1.0
    cosF = cos1.astype(ml_dtypes.bfloat16)
    sinF = sin1.astype(ml_dtypes.bfloat16)
    a = np.arange(P)
    tri = (a[None, :] >= a[:, None]).astype(ml_dtypes.bfloat16)  # [tk, tq]
    idn = np.eye(P, dtype=ml_dtypes.bfloat16)
    smat = np.zeros((P, P), dtype=ml_dtypes.bfloat16)
    smat[np.arange(H) + H, np.arange(H)] = 1     # out[i<64]  = q[i+64]
    smat[np.arange(H), np.arange(H) + H] = 1     # out[i>=64] = q[i-64]
    return cosF, sinF, tri, idn, smat


TRACE = False
LAST_EXEC_NS = None
LAST_TRACE = None
LAST_INSTS = None


def kernel(x, W_qkv, sqk):
    global LAST_EXEC_NS, LAST_TRACE, LAST_INSTS
    T = x.shape[1]
    NTB = T // TB
    cosF, sinF, tri, idn, smat = _host_tables(T)
    WT = np.ascontiguousarray(np.asarray(W_qkv).T).astype(ml_dtypes.bfloat16)
    sqk232 = ((C ** 0.5) * np.asarray(sqk, np.float64) ** 2).astype(
        np.float32).reshape(D, 1)
    onb = np.ones((P, 1), ml_dtypes.bfloat16)
    in_maps = []
    for b in range(B):
        xb = np.asarray(x[b]).T.astype(ml_dtypes.bfloat16)   # [C, T]
        xb = np.ascontiguousarray(
            xb.reshape(NCO, P, NTB, TB).transpose(1, 2, 0, 3))
        in_maps.append({
            "xT": xb,
            "WT": WT,
            "cosF": cosF,
            "sinF": sinF,
            "tri": tri,
            "idn": idn,
            "smat": smat,
            "onb": onb,
            "sqk232": sqk232,
        })
    nc = build_nc(T=T, num_devices=B)
    res = run_bass_kernel_spmd(nc, in_maps, core_ids=list(range(B)),
                               trace=TRACE)
    LAST_EXEC_NS = res.exec_time_ns
    LAST_TRACE = (res.instructions_and_trace[1]
                  if res.instructions_and_trace else None)
    LAST_INSTS = (res.instructions_and_trace[0]
                  if res.instructions_and_trace else None)
    out = np.stack([r["outT"].T for r in res.results])  # [B, T, D]
    return np.ascontiguousarray(out).astype(np.float32)


# revision 6
# speedup vs baseline: 1.5871x; 1.5871x over previous
"""Trainium2 Bass kernel for a single nGPT-style attention head.

Computation (see reference): fused QKV projection, RoPE over the full head
dim, L2-normalize q/k scaled by sqk, causal SDPA with scale sqrt(d_model).

Sharding: data-parallel over batch - 8 batch elements, one per NeuronCore.

Per-core layout: everything transposed, [d, t] with head/feature dim on
SBUF partitions. Software-pipelined per-block structure (TB=512 token
blocks): block j runs QKV matmuls, norms, RoPE, while the ATTENTION of
query block j-1 is interleaved strip-by-strip between the QKV matmul
groups so the scores-exp latency hides behind projection matmuls.

Engine assignment (keeps the scalar engine's activation table resident on
the exp-containing set for the whole kernel - a single ACT_TABLE_LOAD):
  PE:  QKV matmuls, rotate-half (permutation matmul), v transpose
       (identity matmul), transposed norm reductions (sq^T @ ones ->
       per-token partition layout), scores, attn@v, denominator (ones)
       accumulation.
  ACT: scores exp only, with the 1/||k|| fold done via the per-partition
       scale AP operand.
  DVE: psum->sbuf copies, squares, Quake-style rsqrt (bitcast + shift
       seed + 2 Newton steps), RoPE elementwise chain, causal triangle
       mask, reciprocal_approx_fast, final out normalize.
  GpSimd: inv-norm partition broadcasts only.
"""

import numpy as np
import ml_dtypes

import concourse.bass as bass
import concourse.tile as tile
from concourse import bacc, mybir
from concourse.bass import ts, ds
from concourse.bass_utils import run_bass_kernel_spmd

# Surface compile-hook exceptions (the PJRT bridge swallows tracebacks).
try:
    import traceback
    import libneuronxla as _lnx

    if not getattr(_lnx, "_err_wrapped", False):
        _orig_cc = _lnx.neuronx_cc

        def _cc_wrapper(*a, **kw):
            try:
                return _orig_cc(*a, **kw)
            except BaseException:
                traceback.print_exc()
                raise

        _lnx.neuronx_cc = _cc_wrapper
        _lnx._err_wrapped = True
except Exception:
    pass

AFT = mybir.ActivationFunctionType
ALU = mybir.AluOpType
F32 = mybir.dt.float32
BF16 = mybir.dt.bfloat16
I32 = mybir.dt.int32

B, T_FULL, C, D = 8, 2048, 1024, 128
ROPE_BASE = 10000.0
P = 128
TB = 512            # token block (tq block width = one PSUM bank of f32)
NCO = C // P        # contraction chunks for the QKV projection
H = P // 2


def build_nc(T=T_FULL, num_devices=8):
    from contextlib import ExitStack
    NTB = T // TB
    NKT = T // P
    nc = bacc.Bacc("TRN2", target_bir_lowering=False, debug=False,
                   num_devices=num_devices)

    xT = nc.dram_tensor("xT", [P, NTB, NCO, TB], BF16,
                        kind="ExternalInput").ap()
    WT = nc.dram_tensor("WT", [C, 3 * D], BF16, kind="ExternalInput").ap()
    cosF = nc.dram_tensor("cosF", [P, T], BF16, kind="ExternalInput").ap()
    sinF = nc.dram_tensor("sinF", [P, T], BF16, kind="ExternalInput").ap()
    tri = nc.dram_tensor("tri", [P, P], BF16, kind="ExternalInput").ap()
    idn = nc.dram_tensor("idn", [P, P], BF16, kind="ExternalInput").ap()
    smat = nc.dram_tensor("smat", [P, P], BF16, kind="ExternalInput").ap()
    onb = nc.dram_tensor("onb", [P, 1], BF16, kind="ExternalInput").ap()
    sqk232 = nc.dram_tensor("sqk232", [D, 1], F32, kind="ExternalInput").ap()
    outT = nc.dram_tensor("outT", [D, T], F32, kind="ExternalOutput").ap()

    WT_t = WT.rearrange("(co p) d -> p co d", p=P)

    with tile.TileContext(nc) as tc:
        with ExitStack() as ctx:
            const = ctx.enter_context(tc.tile_pool(name="const", bufs=1))
            work = ctx.enter_context(tc.tile_pool(name="work", bufs=2))
            xpool = ctx.enter_context(tc.tile_pool(name="xpool", bufs=NTB))
            expool = ctx.enter_context(tc.tile_pool(name="expool", bufs=4))
            ps_big = ctx.enter_context(
                tc.tile_pool(name="ps_big", bufs=2, space="PSUM"))
            ps_sc = ctx.enter_context(
                tc.tile_pool(name="ps_sc", bufs=3, space="PSUM"))
            ps_o = ctx.enter_context(
                tc.tile_pool(name="ps_o", bufs=1, space="PSUM"))
            ps_d = ctx.enter_context(
                tc.tile_pool(name="ps_d", bufs=1, space="PSUM"))
            ps_sm = ctx.enter_context(
                tc.tile_pool(name="ps_sm", bufs=1, space="PSUM"))

            # constants (small first, then the big streams in need order)
            tri_sb = const.tile([P, P], BF16)
            nc.sync.dma_start(tri_sb, tri)
            idn_sb = const.tile([P, P], BF16)
            nc.sync.dma_start(idn_sb, idn)
            smat_sb = const.tile([P, P], BF16)
            nc.sync.dma_start(smat_sb, smat)
            onb_sb = const.tile([P, 1], BF16)
            nc.sync.dma_start(onb_sb, onb)
            sqk_sb = const.tile([D, 1], F32)
            nc.sync.dma_start(sqk_sb, sqk232)
            wt = const.tile([P, NCO, 3 * D], BF16)
            nc.sync.dma_start(wt, WT_t)

            xts = []
            for j in range(NTB):
                xt = xpool.tile([P, NCO, TB], BF16, tag="xt", name=f"xt{j}")
                nc.sync.dma_start(xt, xT[:, j])
                xts.append(xt)
                if j == 0:
                    cos_sb = const.tile([P, T], BF16)
                    nc.sync.dma_start(cos_sb, cosF)
                    sin_sb = const.tile([P, T], BF16)
                    nc.sync.dma_start(sin_sb, sinF)

            qk = const.tile([P, 2 * T], BF16)    # roped q^T | roped-raw k^T
            vt = const.tile([P, NKT, P], BF16)   # v tiles [tk, e]
            invk = const.tile([P, NKT], F32)     # 1/||k|| per k-strip column

            # ---- pending attention work queue (strips of block J=j-1) ----
            pend = []

            def drain(n):
                for _ in range(min(n, len(pend))):
                    pend.pop(0)()

            def make_att(J):
                """Queue attention for query block J as strip closures."""
                q_blk = qk[:, ts(J, TB)]
                po = ps_o.tile([P, TB], F32, tag="po", name=f"po{J}")
                pd = ps_d.tile([1, TB], F32, tag="pd", name=f"pd{J}")
                nstr = 4 * (J + 1)
                exs = {}

                def emit_scores(i):
                    dr = i - 4 * J
                    off = P * dr if dr >= 0 else 0
                    w = TB - off
                    sc = ps_sc.tile([P, TB], F32, tag="sc",
                                    name=f"sc{J}i{i}")
                    nc.tensor.matmul(sc[:, ds(off, w)],
                                     qk[:, ds(T + P * i, P)],
                                     q_blk[:, ds(off, w)],
                                     start=True, stop=True)
                    ex = expool.tile([P, TB], BF16, tag="ex",
                                     name=f"ex{J}i{i}")
                    nc.scalar.activation(ex[:, ds(off, w)],
                                         sc[:, ds(off, w)], AFT.Exp,
                                         scale=invk[:, ds(i, 1)])
                    if dr >= 0:
                        nc.vector.tensor_mul(ex[:, ds(off, P)],
                                             ex[:, ds(off, P)], tri_sb)
                    exs[i] = (ex, off)

                def emit_av(i):
                    ex, off = exs.pop(i)
                    w = TB - off
                    nc.tensor.matmul(po[:, ds(off, w)], vt[:, i],
                                     ex[:, ds(off, w)],
                                     start=(i == 0), stop=(i == nstr - 1))
                    nc.tensor.matmul(pd[:, ds(off, w)], onb_sb,
                                     ex[:, ds(off, w)],
                                     start=(i == 0), stop=(i == nstr - 1))

                def fin():
                    with nc.named_scope(f"fin{J}"):
                        invd = work.tile([1, TB], F32, tag="invd")
                        nc.vector.reciprocal_approx_fast(out=invd, in_=pd)
                        bcd = work.tile([P, TB], F32, tag="bcd")
                        nc.gpsimd.partition_broadcast(bcd, invd)
                        ob = work.tile([P, TB], F32, tag="ob")
                        nc.vector.tensor_mul(ob, po, bcd)
                        nc.sync.dma_start(outT[:, ts(J, TB)], ob)

                def strip(i, J=J):
                    def run():
                        with nc.named_scope(f"att{J}s{i}"):
                            emit_scores(i)
                            if i >= 2:
                                emit_av(i - 2)
                    return run

                def last():
                    with nc.named_scope(f"att{J}tail"):
                        emit_av(nstr - 2)
                        emit_av(nstr - 1)
                        fin()

                for i in range(nstr):
                    pend.append(strip(i))
                pend.append(last)

            for j in range(NTB):
                xt = xts[j]
                # ---------------- QKV projection + squares ----------------
                with nc.named_scope(f"qkv{j}"):
                    sq = work.tile([P, 2, TB], BF16, tag="sq")
                    for g in range(3):
                        ps = ps_big.tile([P, TB], F32, tag="big",
                                         name=f"qkv{j}g{g}")
                        for co in range(NCO):
                            nc.tensor.matmul(
                                ps, wt[:, co, ts(g, D)], xt[:, co],
                                start=(co == 0), stop=(co == NCO - 1))
                            if co == 3:
                                drain(1)
                        if g < 2:
                            dst = qk[:, ds(g * T + j * TB, TB)]
                            nc.vector.tensor_copy(dst, ps)
                            nc.vector.tensor_mul(sq[:, g], dst, dst)
                        else:
                            vst = work.tile([P, TB], BF16, tag="vst")
                            nc.vector.tensor_copy(vst, ps)
                        drain(1)

                    # transposed norm reductions: nrm[t_local, g*4+c]
                    nrm = ps_sm.tile([P, 8], F32, tag="small",
                                     name=f"nrm{j}")
                    for g in range(2):
                        for c in range(4):
                            nc.tensor.matmul(
                                nrm[:, ds(g * 4 + c, 1)],
                                sq[:, g, ts(c, P)], onb_sb,
                                start=True, stop=True)
                    nrs = work.tile([P, 8], F32, tag="nrs")
                    nc.vector.tensor_copy(nrs, nrm)
                    drain(1)

                    # v transpose via identity matmul
                    for c in range(4):
                        tp = ps_sm.tile([P, P], BF16, tag="small",
                                        name=f"vtp{j}c{c}")
                        nc.tensor.transpose(tp, vst[:, ts(c, P)], idn_sb)
                        nc.vector.tensor_copy(vt[:, 4 * j + c], tp)
                        drain(1)

                # -------- rsqrt of norms (Quake seed + 2 Newton) on DVE ----
                with nc.named_scope(f"nrm{j}"):
                    hg = work.tile([P, 8], I32, tag="hg")
                    nc.vector.tensor_scalar(
                        out=hg, in0=nrs.bitcast(I32), scalar1=1,
                        scalar2=None, op0=ALU.logical_shift_right)
                    nc.vector.tensor_scalar(
                        out=hg, in0=hg, scalar1=-1.0,
                        scalar2=float(0x5F3759DF), op0=ALU.mult, op1=ALU.add)
                    y = work.tile([P, 8], F32, tag="y")
                    nc.vector.tensor_copy(y, hg.bitcast(F32))
                    a = work.tile([P, 8], F32, tag="a")
                    nc.vector.tensor_mul(a, y, y)
                    nc.vector.tensor_mul(a, a, nrs)
                    nc.vector.tensor_scalar(out=a, in0=a, scalar1=-0.5,
                                            scalar2=1.5, op0=ALU.mult,
                                            op1=ALU.add)
                    nc.vector.tensor_mul(y, y, a)
                    nc.vector.tensor_mul(a, y, y)
                    nc.vector.tensor_mul(a, a, nrs)
                    nc.vector.tensor_scalar(out=a, in0=a, scalar1=-0.5,
                                            scalar2=1.5, op0=ALU.mult,
                                            op1=ALU.add)
                    yqb = work.tile([P, 4], BF16, tag="yqb")
                    nc.vector.tensor_mul(yqb, y[:, 0:4], a[:, 0:4])
                    nc.vector.tensor_mul(invk[:, ds(4 * j, 4)],
                                         y[:, 4:8], a[:, 4:8])
                    drain(1)

                    # 1/||q||: transpose each [128,1] column to a
                    # partition-0 row of a [1,TB] staging tile, broadcast
                    # across partitions on GpSimd.
                    iqt = work.tile([1, TB], F32, tag="iqt")
                    for c in range(4):
                        tq = ps_sm.tile([1, P], BF16, tag="small",
                                        name=f"tq{j}c{c}")
                        nc.tensor.transpose(tq, yqb[:, ds(c, 1)], idn_sb)
                        nc.vector.tensor_copy(iqt[:, ts(c, P)], tq)
                    bcq = work.tile([P, TB], F32, tag="bcq")
                    nc.gpsimd.partition_broadcast(bcq, iqt)
                    drain(1)

                # ------------------------- RoPE ---------------------------
                with nc.named_scope(f"rope{j}"):
                    ch_t = ds(j * TB, TB)
                    for part in range(2):  # 0 = q, 1 = k
                        chq = ds(part * T + j * TB, TB)
                        rot = ps_big.tile([P, TB], F32, tag="big",
                                          name=f"rot{j}p{part}")
                        nc.tensor.matmul(rot, smat_sb, qk[:, chq],
                                         start=True, stop=True)
                        t2 = work.tile([P, TB], BF16, tag="t2")
                        nc.vector.tensor_mul(t2, rot, sin_sb[:, ch_t])
                        t1 = work.tile([P, TB], BF16, tag="t1")
                        nc.vector.tensor_mul(t1, qk[:, chq], cos_sb[:, ch_t])
                        if part == 0:
                            nc.vector.tensor_add(t1, t1, t2)
                            nc.vector.scalar_tensor_tensor(
                                out=qk[:, chq], in0=t1, scalar=sqk_sb,
                                in1=bcq, op0=ALU.mult, op1=ALU.mult)
                        else:
                            nc.vector.tensor_add(qk[:, chq], t1, t2)
                        drain(1)

                # queue this block's attention; last block drains everything
                make_att(j)
                if j == NTB - 1:
                    drain(len(pend))

    nc.compile()
    return nc


def _host_tables(T):
    d = D
    inv_freq = 1.0 / (ROPE_BASE ** (np.arange(0, d, 2, dtype=np.float64) / d))
    t = np.arange(T, dtype=np.float64)
    freqs = np.outer(inv_freq, t)                 # [d/2, T]
    emb = np.concatenate([freqs, freqs], axis=0)  # [d, T]
    cos1 = np.cos(emb)
    sin1 = np.sin(emb)
    # rotate_half sign folded into the sin table: the device rot is a plain
    # half-swap permutation; sin rows 0:d/2 carry the minus sign.
    sin1[: d // 2, :] *= -1.0
    cosF = cos1.astype(ml_dtypes.bfloat16)
    sinF = sin1.astype(ml_dtypes.bfloat16)
    a = np.arange(P)
    tri = (a[None, :] >= a[:, None]).astype(ml_dtypes.bfloat16)  # [tk, tq]
    idn = np.eye(P, dtype=ml_dtypes.bfloat16)
    smat = np.zeros((P, P), dtype=ml_dtypes.bfloat16)
    smat[np.arange(H) + H, np.arange(H)] = 1     # out[i<64]  = q[i+64]
    smat[np.arange(H), np.arange(H) + H] = 1     # out[i>=64] = q[i-64]
    return cosF, sinF, tri, idn, smat


TRACE = False
LAST_EXEC_NS = None
LAST_TRACE = None
LAST_INSTS = None


def kernel(x, W_qkv, sqk):
    global LAST_EXEC_NS, LAST_TRACE, LAST_INSTS
    T = x.shape[1]
    NTB = T // TB
    cosF, sinF, tri, idn, smat = _host_tables(T)
    WT = np.ascontiguousarray(np.asarray(W_qkv).T).astype(ml_dtypes.bfloat16)
    sqk232 = ((C ** 0.5) * np.asarray(sqk, np.float64) ** 2).astype(
        np.float32).reshape(D, 1)
    onb = np.ones((P, 1), ml_dtypes.bfloat16)
    in_maps = []
    for b in range(B):
        xb = np.asarray(x[b]).T.astype(ml_dtypes.bfloat16)   # [C, T]
        xb = np.ascontiguousarray(
            xb.reshape(NCO, P, NTB, TB).transpose(1, 2, 0, 3))
        in_maps.append({
            "xT": xb,
            "WT": WT,
            "cosF": cosF,
            "sinF": sinF,
            "tri": tri,
            "idn": idn,
            "smat": smat,
            "onb": onb,
            "sqk232": sqk232,
        })
    nc = build_nc(T=T, num_devices=B)
    res = run_bass_kernel_spmd(nc, in_maps, core_ids=list(range(B)),
                               trace=TRACE)
    LAST_EXEC_NS = res.exec_time_ns
    LAST_TRACE = (res.instructions_and_trace[1]
                  if res.instructions_and_trace else None)
    LAST_INSTS = (res.instructions_and_trace[0]
                  if res.instructions_and_trace else None)
    out = np.stack([r["outT"].T for r in res.results])  # [B, T, D]
    return np.ascontiguousarray(out).astype(np.float32)
